# Initial kernel scaffold
#
"""Trainium2 Bass kernel: MeanFieldMultiDimensionalLogisticRegression.

Computes, for X:[N,D], z:[S], w_mu:[D], w_log_var:[D]:
    mean_i = X @ w_mu                       [N]
    var_i  = sum(X^2 * exp(w_log_var), -1)  [N]
    act    = std_i[:,None]*z[None,:] + mean_i[:,None]   [N,S]
    Y      = sigmoid(act)
returns (Y, act).

Data-parallel over 8 NeuronCores: X (and outputs) sharded along N;
w_mu / w_log_var / z replicated (pre-broadcast to 128 partitions on host).

Per-core device program (rows_per_core=2048, 16 tiles of 128 rows):
  phase A (per tile): DMA X tile -> DVE tensor_tensor_reduce (mean)
      -> DVE mult by sqrt(exp(w_log_var)) -> ACT square+row-accum (var)
  std = ACT sqrt(var) once for all 16 tiles
  phase B (per tile): DVE tensor_scalar act = z*std+mean
      -> ACT sigmoid -> DMA store both outputs.
"""

import os
import numpy as np

import concourse.bass as bass
import concourse.tile as tile
from concourse import mybir
from concourse.bass_utils import run_bass_kernel_spmd

N, D, S = 16384, 1024, 256
NCORES = 8
NSHARD = N // NCORES  # 2048 rows per core
P = 128               # SBUF partitions
NT = NSHARD // P      # 16 row-tiles per core
F32 = mybir.dt.float32

_cached_nc = None
last_result = None  # BassKernelResults of the most recent run (for test harness)


def build_program():
    """Build the per-core Bass/Tile program (identical on all 8 cores)."""
    nc = bass.Bass("TRN2", debug=False, num_devices=NCORES)

    x_h = nc.declare_dram_parameter("x", [NSHARD, D], F32, isOutput=False)
    wb_h = nc.declare_dram_parameter("wb", [P, D], F32, isOutput=False)
    qb_h = nc.declare_dram_parameter("qb", [P, D], F32, isOutput=False)
    zb_h = nc.declare_dram_parameter("zb", [P, S], F32, isOutput=False)
    act_h = nc.declare_dram_parameter("act", [NSHARD, S], F32, isOutput=True)
    y_h = nc.declare_dram_parameter("y", [NSHARD, S], F32, isOutput=True)

    AF = mybir.ActivationFunctionType
    OP = mybir.AluOpType

    with tile.TileContext(nc) as tc:
        with (
            tc.tile_pool(name="consts", bufs=1) as consts,
            tc.tile_pool(name="xp", bufs=4) as xp,
            tc.tile_pool(name="xsp", bufs=3) as xsp,
            tc.tile_pool(name="stats", bufs=1) as stats,
            tc.tile_pool(name="outp", bufs=4) as outp,
        ):
            wb = consts.tile([P, D], F32)  # w_mu broadcast along partitions
            nc.sync.dma_start(out=wb[:], in_=wb_h[:])
            qb = consts.tile([P, D], F32)  # sqrt(exp(w_log_var)) broadcast
            nc.sync.dma_start(out=qb[:], in_=qb_h[:])
            zb = consts.tile([P, S], F32)  # z broadcast
            nc.sync.dma_start(out=zb[:], in_=zb_h[:])

            mean_all = stats.tile([P, NT], F32)
            var_all = stats.tile([P, NT], F32)
            std_all = stats.tile([P, NT], F32)

            for t in range(NT):
                xt = xp.tile([P, D], F32)
                nc.sync.dma_start(out=xt[:], in_=x_h[t * P:(t + 1) * P, :])
                xs = xsp.tile([P, D], F32)
                # mean_t = rowsum(X * w_mu); `out` is scratch, overwritten below
                nc.vector.tensor_tensor_reduce(
                    out=xs[:], in0=xt[:], in1=wb[:], scale=1.0, scalar=0.0,
                    op0=OP.mult, op1=OP.add, accum_out=mean_all[:, t:t + 1])
                # xs = X * sqrt(exp(w_log_var))
                nc.vector.tensor_mul(xs[:], xt[:], qb[:])
                # var_t = rowsum(xs^2); square in place on the scalar engine
                nc.scalar.activation(xs[:], xs[:], AF.Square,
                                     accum_out=var_all[:, t:t + 1])

            nc.scalar.activation(std_all[:], var_all[:], AF.Sqrt)

            for t in range(NT):
                at = outp.tile([P, S], F32)
                nc.vector.tensor_scalar(
                    out=at[:], in0=zb[:],
                    scalar1=std_all[:, t:t + 1], scalar2=mean_all[:, t:t + 1],
                    op0=OP.mult, op1=OP.add)
                yt = outp.tile([P, S], F32)
                nc.scalar.activation(yt[:], at[:], AF.Sigmoid)
                nc.sync.dma_start(out=act_h[t * P:(t + 1) * P, :], in_=at[:])
                nc.sync.dma_start(out=y_h[t * P:(t + 1) * P, :], in_=yt[:])

    return nc


def _get_nc():
    global _cached_nc
    if _cached_nc is None:
        _cached_nc = build_program()
    return _cached_nc


def make_host_inputs(X, z, w_mu, w_log_var):
    """Host-side prep: broadcast the small vectors to 128 partitions."""
    X = np.ascontiguousarray(np.asarray(X, dtype=np.float32))
    z = np.asarray(z, dtype=np.float32)
    w_mu = np.asarray(w_mu, dtype=np.float32)
    w_log_var = np.asarray(w_log_var, dtype=np.float32)
    sqew = np.exp(0.5 * w_log_var).astype(np.float32)  # sqrt(exp(w_log_var))
    wb = np.ascontiguousarray(np.broadcast_to(w_mu, (P, D)))
    qb = np.ascontiguousarray(np.broadcast_to(sqew, (P, D)))
    zb = np.ascontiguousarray(np.broadcast_to(z, (P, S)))
    in_maps = [
        {"x": X[k * NSHARD:(k + 1) * NSHARD], "wb": wb, "qb": qb, "zb": zb}
        for k in range(NCORES)
    ]
    return in_maps


def kernel(X, z, w_mu, w_log_var):
    global last_result
    nc = _get_nc()
    in_maps = make_host_inputs(X, z, w_mu, w_log_var)
    trace = bool(int(os.environ.get("KTRACE", "0")))
    res = run_bass_kernel_spmd(nc, in_maps, list(range(NCORES)), trace=trace)
    last_result = res
    Y = np.concatenate([r["y"] for r in res.results], axis=0)
    act = np.concatenate([r["act"] for r in res.results], axis=0)
    return (Y, act)


# revision 6
# speedup vs baseline: 1.4826x; 1.4826x over previous
"""Trainium2 Bass kernel: MeanFieldMultiDimensionalLogisticRegression.

Computes, for X:[N,D], z:[S], w_mu:[D], w_log_var:[D]:
    mean_i = X @ w_mu                       [N]
    var_i  = sum(X^2 * exp(w_log_var), -1)  [N]
    act    = std_i[:,None]*z[None,:] + mean_i[:,None]   [N,S]
    Y      = sigmoid(act)
returns (Y, act).

Data-parallel over 8 NeuronCores: X (and outputs) sharded along N;
w_mu / w_log_var / z replicated (pre-broadcast to 128 partitions on host).

Per-core device program (rows_per_core=2048, 16 tiles of 128 rows):
  phase A (per tile): DMA X tile -> DVE tensor_tensor_reduce (mean)
      -> DVE mult by sqrt(exp(w_log_var)) -> ACT square+row-accum (var)
  std = ACT sqrt(var) once for all 16 tiles
  phase B (per tile): DVE tensor_scalar act = z*std+mean
      -> ACT sigmoid -> DMA store both outputs.
"""

import os
import numpy as np

import concourse.bass as bass
import concourse.tile as tile
from concourse import bacc, mybir
from concourse.bass_utils import run_bass_kernel_spmd

N, D, S = 16384, 1024, 256
NCORES = 8
NSHARD = N // NCORES  # 2048 rows per core
P = 128               # SBUF partitions
NT = NSHARD // P      # 16 row-tiles per core
F32 = mybir.dt.float32

_cached_nc = None
last_result = None  # BassKernelResults of the most recent run (for test harness)


def build_program(reps=1):
    """Build the per-core Bass/Tile program (identical on all 8 cores).

    reps>1 wraps the whole computation in an on-device For_i loop --
    used only for benchmarking (wall-clock slope vs reps)."""
    nc = bacc.Bacc("TRN2", debug=False, num_devices=NCORES)

    x_h = nc.declare_dram_parameter("x", [NSHARD, D], F32, isOutput=False)
    wb_h = nc.declare_dram_parameter("wb", [P, D], F32, isOutput=False)
    qb_h = nc.declare_dram_parameter("qb", [P, D], F32, isOutput=False)
    zb_h = nc.declare_dram_parameter("zb", [P, S], F32, isOutput=False)
    act_h = nc.declare_dram_parameter("act", [NSHARD, S], F32, isOutput=True)
    y_h = nc.declare_dram_parameter("y", [NSHARD, S], F32, isOutput=True)

    AF = mybir.ActivationFunctionType
    OP = mybir.AluOpType

    with tile.TileContext(nc) as tc:
        with (
            tc.tile_pool(name="consts", bufs=1) as consts,
            tc.tile_pool(name="xp", bufs=4) as xp,
            tc.tile_pool(name="xsp", bufs=3) as xsp,
            tc.tile_pool(name="stats", bufs=1) as stats,
            tc.tile_pool(name="outp", bufs=4) as outp,
        ):
            wb = consts.tile([P, D], F32)  # w_mu broadcast along partitions
            nc.sync.dma_start(out=wb[:], in_=wb_h[:])
            qb = consts.tile([P, D], F32)  # sqrt(exp(w_log_var)) broadcast
            nc.sync.dma_start(out=qb[:], in_=qb_h[:])
            zb = consts.tile([P, S], F32)  # z broadcast
            nc.sync.dma_start(out=zb[:], in_=zb_h[:])

            mean_all = stats.tile([P, NT], F32)
            var_all = stats.tile([P, NT], F32)
            std_all = stats.tile([P, NT], F32)

            def body():
                for t in range(NT):
                    xt = xp.tile([P, D], F32)
                    nc.sync.dma_start(out=xt[:], in_=x_h[t * P:(t + 1) * P, :])
                    xs = xsp.tile([P, D], F32)
                    # mean_t = rowsum(X*w_mu); `out` is scratch, overwritten
                    nc.vector.scalar_tensor_tensor(
                        out=xs[:], in0=xt[:], scalar=1.0, in1=wb[:],
                        op0=OP.mult, op1=OP.mult,
                        accum_out=mean_all[:, t:t + 1])
                    # xs = X * sqrt(exp(w_log_var))
                    nc.vector.tensor_mul(xs[:], xt[:], qb[:])
                    # var_t = rowsum(xs^2); square in place on scalar engine
                    nc.scalar.activation(xs[:], xs[:], AF.Square,
                                         accum_out=var_all[:, t:t + 1])

                nc.scalar.activation(std_all[:], var_all[:], AF.Sqrt)

                for t in range(NT):
                    at = outp.tile([P, S], F32)
                    nc.vector.tensor_scalar(
                        out=at[:], in0=zb[:],
                        scalar1=std_all[:, t:t + 1],
                        scalar2=mean_all[:, t:t + 1],
                        op0=OP.mult, op1=OP.add)
                    yt = outp.tile([P, S], F32)
                    nc.scalar.activation(yt[:], at[:], AF.Sigmoid)
                    nc.sync.dma_start(out=act_h[t * P:(t + 1) * P, :],
                                      in_=at[:])
                    nc.sync.dma_start(out=y_h[t * P:(t + 1) * P, :], in_=yt[:])

            if reps == 1:
                body()
            else:
                with tc.For_i(0, reps, 1):
                    body()

    nc.compile()
    return nc


def _get_nc():
    global _cached_nc
    if _cached_nc is None:
        _cached_nc = build_program()
    return _cached_nc


def make_host_inputs(X, z, w_mu, w_log_var):
    """Host-side prep: broadcast the small vectors to 128 partitions."""
    X = np.ascontiguousarray(np.asarray(X, dtype=np.float32))
    z = np.asarray(z, dtype=np.float32)
    w_mu = np.asarray(w_mu, dtype=np.float32)
    w_log_var = np.asarray(w_log_var, dtype=np.float32)
    sqew = np.exp(0.5 * w_log_var).astype(np.float32)  # sqrt(exp(w_log_var))
    wb = np.ascontiguousarray(np.broadcast_to(w_mu, (P, D)))
    qb = np.ascontiguousarray(np.broadcast_to(sqew, (P, D)))
    zb = np.ascontiguousarray(np.broadcast_to(z, (P, S)))
    in_maps = [
        {"x": X[k * NSHARD:(k + 1) * NSHARD], "wb": wb, "qb": qb, "zb": zb}
        for k in range(NCORES)
    ]
    return in_maps


def kernel(X, z, w_mu, w_log_var):
    global last_result
    nc = _get_nc()
    in_maps = make_host_inputs(X, z, w_mu, w_log_var)
    trace = bool(int(os.environ.get("KTRACE", "0")))
    res = run_bass_kernel_spmd(nc, in_maps, list(range(NCORES)), trace=trace)
    last_result = res
    Y = np.concatenate([r["y"] for r in res.results], axis=0)
    act = np.concatenate([r["act"] for r in res.results], axis=0)
    return (Y, act)
